# revision 40
# baseline (speedup 1.0000x reference)
"""Trainium2 Bass kernel for nn_Dist (retrieval_knn): for every pixel of a
B=2 x 64 x 192 grid, find the 4 nearest valid (sparse-depth) pixels, with
jax.lax.top_k tie-breaking (equal distance -> lower pixel index first).

Strategy
--------
Host marshaling: compact the ~5% valid candidate pixels per batch (order-
preserving, so candidate order == pixel-index order). Queries are tiled
into 16x8-pixel blocks (128 queries = 128 partitions); each block gets a
rectangular candidate window (the largest centered rectangle holding at
most 128 candidates). Sharding: 8 cores = 2 batches x 4 row-quarters
(3072 queries = 24 blocks per core).

Device (per core, one Bass/Tile program, SPMD):
  - One K=4 fp32 matmul per block builds the packed selection key:
      nkey[p,s] = KC2 - 256*dist2(p,s) - j_s            (exact integers)
    via centered/scaled coords so every partial sum stays < 2^24 (bit-
    exact fp32; verified on HW). Larger nkey == nearer; ties broken by
    lower pixel index (j follows candidate order within the window).
  - 4 selection rounds on the Vector engine per block:
      m_r = reduce_max(nkey);  nkey = (nkey < m_r) * nkey
    (keys are distinct positive, so masking the extracted max to zero is
    safe; duplicated padding slots share one key and collapse together).
  - Output: the 4 winning keys per query, decoded on host into indices
    and offsets (pure elementwise output formatting).

Correctness for arbitrary inputs: after the run the host *proves* the
windowed result exact: the 4th-nearest squared distance returned for each
query must be strictly below the squared distance to the nearest grid row/
column not covered by the window rectangle. Any failing queries (none, for
typical 5%-density inputs) are recomputed exactly.
"""

import sys

sys.path.insert(0, "/opt/trn_rl_repo")

import numpy as np

B = 2
H = 64
W = 192
N = H * W            # 12288 pixels
NUM = 4
WIN = 48             # candidates per window (j fits in 8 bits)
GROUP_SIZES = (8, 8, 8)  # tiles per group (8*WIN == one PSUM bank)
KROWS = 14           # bf16 split-rows per tile (exact 8-bit-mantissa parts)
GK = KROWS * max(GROUP_SIZES)  # block-structured contraction dim per group
MM_SPLIT = 8           # tiles per matmul within a group (8*WIN <= PSUM bank)
KC2 = 10_356_000     # key bias: > 256*max_dist2 + 255, keeps partials < 2^24
N_CORES = 8
QPC = N // 4         # queries per core (4 cores per batch)
TILES = QPC // 128   # 24 blocks of 16x8 queries
BX, BY = 16, 8       # block extent in x / y
NBX = W // BX        # 12 blocks across
VALID_THRESH = 0.001

_PROGRAM = None


def _build_program():
    import concourse.bass as bass
    import concourse.bacc as bacc
    import concourse.mybir as mybir
    from concourse.tile import TileContext

    f32 = mybir.dt.float32
    ngroups = len(GROUP_SIZES)
    nc = bacc.Bacc(None)
    bf16 = mybir.dt.bfloat16
    lhsT_in = nc.declare_dram_parameter("lhsT_in", [GK, ngroups * 128], bf16, isOutput=False)
    rhs_in = nc.declare_dram_parameter("rhs_in", [GK, TILES * WIN], bf16, isOutput=False)
    m_out = nc.declare_dram_parameter("m_out", [128, TILES * NUM], f32, isOutput=True)

    with TileContext(nc) as tc:
        with (
            tc.tile_pool(name="const", bufs=1) as const,
            tc.tile_pool(name="work", bufs=2) as work,
            tc.tile_pool(name="psum", bufs=3, space="PSUM") as psum,
        ):
            lhsT_sb = const.tile([GK, ngroups * 128], bf16)
            rhs_sb = const.tile([GK, TILES * WIN], bf16)
            # mr layout: group-major, then round, then tile-in-group:
            # col = gbase*NUM + r*gsz + i
            mr = const.tile([128, TILES * NUM], f32)
            # group 0's inputs first (fast matmul start), then the rest
            # as single streaming transfers (fewer descriptor rounds)
            g0 = GROUP_SIZES[0]
            nc.sync.dma_start(out=rhs_sb[:, : g0 * WIN], in_=rhs_in[:, : g0 * WIN])
            nc.sync.dma_start(out=lhsT_sb[:, :128], in_=lhsT_in[:, :128])
            nc.sync.dma_start(out=rhs_sb[:, g0 * WIN :], in_=rhs_in[:, g0 * WIN :])
            nc.sync.dma_start(out=lhsT_sb[:, 128:], in_=lhsT_in[:, 128:])

            # shared contiguous chain tiles; rounds 0-1 run per group (they
            # overlap the DMA/matmul ramp), rounds 2-3 run once over all 24
            # tiles (amortized op overhead; by then the ramp is done).
            # mr layout: col = r*TILES + t
            AW = TILES * WIN
            nk1 = const.tile([128, AW], f32)
            nk2 = const.tile([128, AW], f32)
            nk3 = const.tile([128, AW], f32)
            msk = const.tile([128, AW], f32)

            def bcast(sl):
                return bass.AP(sl.tensor, sl.offset, list(sl.ap) + [[0, WIN]])

            def v3(tile_ap):
                return tile_ap.rearrange("p (t w) -> p t w", w=WIN)

            gb = 0
            for g, gsz in enumerate(GROUP_SIZES):
                assert gsz <= MM_SPLIT
                GW = gsz * WIN
                acc = psum.tile([128, GW], f32, tag="acc")
                nc.tensor.matmul(
                    acc[:, :],
                    lhsT_sb[:, g * 128 : (g + 1) * 128],
                    rhs_sb[:, gb * WIN : (gb + gsz) * WIN],
                    start=True,
                    stop=True,
                )
                w0, w1 = gb * WIN, (gb + gsz) * WIN
                # round 0 straight from PSUM (no copy), round 1 from SBUF
                for r, (src_t, dst_t) in enumerate([(acc[:, :], nk1), (nk1[:, w0:w1], nk2)]):
                    col = r * TILES + gb
                    nc.vector.tensor_reduce(
                        out=mr[:, col : col + gsz],
                        in_=v3(src_t),
                        axis=mybir.AxisListType.X,
                        op=mybir.AluOpType.max,
                    )
                    nc.vector.tensor_tensor(
                        out=v3(msk[:, w0:w1]),
                        in0=v3(src_t),
                        in1=bcast(mr[:, col : col + gsz]),
                        op=mybir.AluOpType.is_lt,
                    )
                    nc.vector.tensor_tensor(
                        out=v3(dst_t[:, w0:w1]),
                        in0=v3(msk[:, w0:w1]),
                        in1=v3(src_t),
                        op=mybir.AluOpType.mult,
                    )
                gb += gsz

            # global rounds 2-3
            col = 2 * TILES
            nc.vector.tensor_reduce(
                out=mr[:, col : col + TILES], in_=v3(nk2[:, :]),
                axis=mybir.AxisListType.X, op=mybir.AluOpType.max,
            )
            nc.vector.tensor_tensor(
                out=v3(msk[:, :]), in0=v3(nk2[:, :]),
                in1=bcast(mr[:, col : col + TILES]), op=mybir.AluOpType.is_lt,
            )
            nc.vector.tensor_tensor(
                out=v3(nk3[:, :]), in0=v3(msk[:, :]), in1=v3(nk2[:, :]),
                op=mybir.AluOpType.mult,
            )
            col = 3 * TILES
            nc.vector.tensor_reduce(
                out=mr[:, col : col + TILES], in_=v3(nk3[:, :]),
                axis=mybir.AxisListType.X, op=mybir.AluOpType.max,
            )
            nc.sync.dma_start(out=m_out[:, :], in_=mr[:, :])

    nc.compile()
    return nc


def _split2(x, lo_bits):
    """x = hi + lo with both parts exact in bf16 (<=8-bit mantissa)."""
    x = np.asarray(x, dtype=np.int64)
    s = np.sign(x)
    hi = s * ((np.abs(x) >> lo_bits) << lo_bits)
    return hi.astype(np.float64), (x - hi).astype(np.float64)


def _split3(x, b1, b2):
    x = np.asarray(x, dtype=np.int64)
    s = np.sign(x)
    p1 = s * ((np.abs(x) >> b1) << b1)
    rem = x - p1
    s2 = np.sign(rem)
    p2 = s2 * ((np.abs(rem) >> b2) << b2)
    return p1.astype(np.float64), p2.astype(np.float64), (rem - p2).astype(np.float64)


def _block_origin(q, t):
    """Grid origin (x0, y0) of block t in row-quarter q."""
    return (t % NBX) * BX, 16 * q + (t // NBX) * BY


def _choose_rect(ps, x0, y0):
    """Largest centered rectangle [x0-R, x0+BX-1+R] x [y0-R, y0+BY-1+R]
    holding at most WIN candidates. ps = 2D prefix sum of the valid mask."""

    def count(R):
        xl, xh = max(x0 - R, 0), min(x0 + BX - 1 + R, W - 1)
        yl, yh = max(y0 - R, 0), min(y0 + BY - 1 + R, H - 1)
        return ps[yh + 1, xh + 1] - ps[yl, xh + 1] - ps[yh + 1, xl] + ps[yl, xl]

    for R in range(40, 3, -1):
        if count(R) <= WIN:
            return R
    return 4


def _exact_topk_host(qn, g, cx, cy):
    """Exact reference-equivalent top-NUM for query pixels qn (fallback for
    queries whose window guarantee failed; not used on typical inputs)."""
    qx = qn % W
    qy = qn // W
    d = (qx[:, None] - cx[None, :]) ** 2 + (qy[:, None] - cy[None, :]) ** 2
    order = np.argsort(d, axis=1, kind="stable")  # ties -> lower slot = lower g
    sel = order[:, :NUM]
    return g[sel]


def kernel(S, xx, yy, num):
    from concourse.bass_utils import run_bass_kernel_spmd

    global _PROGRAM
    S = np.asarray(S)
    num = int(num)
    assert num == NUM and S.shape == (B, 1, H, W)

    Sf = S.reshape(B, N)
    valid = Sf > VALID_THRESH

    g_all, cx_all, cy_all, ps_all = [], [], [], []
    for b in range(B):
        g = np.nonzero(valid[b])[0].astype(np.int64)
        if len(g) < NUM:
            raise NotImplementedError("fewer than 4 valid candidates")
        g_all.append(g)
        cx_all.append(g % W)
        cy_all.append(g // W)
        ps = np.zeros((H + 1, W + 1), dtype=np.int64)
        ps[1:, 1:] = valid[b].reshape(H, W).cumsum(0).cumsum(1)
        ps_all.append(ps)

    # block-local partition -> query offset maps
    px = np.arange(128) % BX
    py = np.arange(128) // BX

    # ---- host marshaling: per-core lhsT / windowed rhs tables ----
    in_maps = []
    winmap = np.zeros((N_CORES, TILES, WIN), dtype=np.int64)  # slot -> pixel
    rects = np.zeros((N_CORES, TILES, 4), dtype=np.int64)     # xl, xh, yl, yh
    # tile -> (group, index-in-group); tiles are consecutive within groups
    grp_of, idx_of = [], []
    for g, gsz in enumerate(GROUP_SIZES):
        grp_of += [g] * gsz
        idx_of += list(range(gsz))
    ngroups = len(GROUP_SIZES)

    for c in range(N_CORES):
        b, q = c // 4, c % 4
        g, cx, cy, ps = g_all[b], cx_all[b], cy_all[b], ps_all[b]
        lhsT = np.zeros((GK, ngroups * 128), dtype=np.float32)
        rhs = np.zeros((GK, TILES * WIN), dtype=np.float32)
        for t in range(TILES):
            x0, y0 = _block_origin(q, t)
            qx = x0 + px
            qy = y0 + py
            a = 16 * qx - 1528
            ay = 16 * qy - 504
            gi, ii = grp_of[t], idx_of[t]
            sl = slice(gi * 128, (gi + 1) * 128)
            v0a, v0b, v0c = _split3(-(a * a + ay * ay), 14, 6)
            ah, al = _split2(a, 3)
            ayh, ayl = _split2(ay, 2)
            lrows = [v0a, v0b, v0c, ah, ah, al, al, ayh, ayh, ayl, ayl,
                     np.ones(128), np.ones(128), np.ones(128)]
            for k, row in enumerate(lrows):
                lhsT[KROWS * ii + k, sl] = row

            R = _choose_rect(ps, x0, y0)
            xl, xh = max(x0 - R, 0), min(x0 + BX - 1 + R, W - 1)
            yl, yh = max(y0 - R, 0), min(y0 + BY - 1 + R, H - 1)
            rects[c, t] = (xl, xh, yl, yh)
            inside = (cx >= xl) & (cx <= xh) & (cy >= yl) & (cy <= yh)
            sel = np.nonzero(inside)[0]          # ascending == g order
            cnt = len(sel)
            src = sel[np.minimum(np.arange(WIN), max(cnt - 1, 0))] if cnt else None
            if cnt == 0:
                bx = np.zeros(WIN); by = np.zeros(WIN)
                jj = np.zeros(WIN); winmap[c, t] = 0
            else:
                bx = 16 * cx[src] - 1528
                by = 16 * cy[src] - 504
                jj = np.minimum(np.arange(WIN), cnt - 1)  # dups share j
                winmap[c, t] = g[src]
            ws = slice(t * WIN, (t + 1) * WIN)
            bh, bl = _split2(2 * bx, 4)
            byh, byl = _split2(2 * by, 2)
            v3a, v3b, v3c = _split3(KC2 - jj - bx * bx - by * by, 16, 8)
            on = np.ones(WIN)
            rrows = [on, on, on, bh, bl, bh, bl, byh, byl, byh, byl, v3a, v3b, v3c]
            for k, row in enumerate(rrows):
                rhs[KROWS * ii + k, ws] = row
        import ml_dtypes
        in_maps.append({"lhsT_in": lhsT.astype(ml_dtypes.bfloat16),
                        "rhs_in": rhs.astype(ml_dtypes.bfloat16)})

    if _PROGRAM is None:
        _PROGRAM = _build_program()
    globals()["_LAST_IN_MAPS"] = in_maps
    res = run_bass_kernel_spmd(_PROGRAM, in_maps, list(range(N_CORES)))

    # ---- decode on host (elementwise output formatting) ----
    args = np.zeros((B, NUM, N), dtype=np.int32)
    gsel = np.zeros((B, NUM, N), dtype=np.int64)
    dist4 = np.zeros((B, N), dtype=np.int64)
    for c in range(N_CORES):
        b, q = c // 4, c % 4
        m_raw = np.rint(res.results[c]["m_out"].astype(np.float64)).astype(np.int64)
        # device layout col = r*TILES + t  ->  [p, t, r]
        colmap = np.empty((TILES, NUM), dtype=np.int64)
        for t in range(TILES):
            for r in range(NUM):
                colmap[t, r] = r * TILES + t
        m = m_raw[:, colmap]
        key = KC2 - m                     # = 256*d + j
        d = key >> 8
        j = np.clip(key & 255, 0, WIN - 1)
        for t in range(TILES):
            x0, y0 = _block_origin(q, t)
            n_of = (y0 + py) * W + (x0 + px)      # [128] pixel index
            gg = winmap[c, t][j[:, t, :]]         # [128, NUM]
            for r in range(NUM):
                args[b, r, n_of] = gg[:, r].astype(np.int32)
                gsel[b, r, n_of] = gg[:, r]
            dist4[b, n_of] = d[:, t, NUM - 1]

    # ---- post-hoc exactness proof of the windowed result ----
    for c in range(N_CORES):
        b, q = c // 4, c % 4
        for t in range(TILES):
            xl, xh, yl, yh = rects[c, t]
            x0, y0 = _block_origin(q, t)
            qx = x0 + px
            qy = y0 + py
            n_of = qy * W + qx
            guard = np.full(128, 10**12, dtype=np.int64)
            if xl > 0:
                guard = np.minimum(guard, (qx - xl + 1) ** 2)
            if xh < W - 1:
                guard = np.minimum(guard, (xh + 1 - qx) ** 2)
            if yl > 0:
                guard = np.minimum(guard, (qy - yl + 1) ** 2)
            if yh < H - 1:
                guard = np.minimum(guard, (yh + 1 - qy) ** 2)
            bad = dist4[b, n_of] >= guard
            if bad.any():
                qbad = n_of[bad]
                gsel_f = _exact_topk_host(qbad, g_all[b], cx_all[b], cy_all[b])
                args[b, :, qbad] = gsel_f.astype(np.int32)
                gsel[b, :, qbad] = gsel_f

    # IPC offsets from the input coordinate grids (matches reference's
    # xy[:, args] - xy[:, query])
    xf = np.asarray(xx, dtype=np.float32).reshape(N)
    yf = np.asarray(yy, dtype=np.float32).reshape(N)
    IPCnum = np.empty((B, 2, NUM, N), dtype=np.float32)
    for b in range(B):
        IPCnum[b, 0] = xf[gsel[b]] - xf[None, :]
        IPCnum[b, 1] = yf[gsel[b]] - yf[None, :]

    return IPCnum, args


# revision 41
# speedup vs baseline: 1.0801x; 1.0801x over previous
"""Trainium2 Bass kernel for nn_Dist (retrieval_knn): for every pixel of a
B=2 x 64 x 192 grid, find the 4 nearest valid (sparse-depth) pixels, with
jax.lax.top_k tie-breaking (equal distance -> lower pixel index first).

Strategy
--------
Host marshaling: compact the ~5% valid candidate pixels per batch (order-
preserving, so candidate order == pixel-index order). Queries are tiled
into 16x8-pixel blocks (128 queries = 128 partitions); each block gets a
rectangular candidate window (the largest centered rectangle holding at
most 128 candidates). Sharding: 8 cores = 2 batches x 4 row-quarters
(3072 queries = 24 blocks per core).

Device (per core, one Bass/Tile program, SPMD):
  - One K=4 fp32 matmul per block builds the packed selection key:
      nkey[p,s] = KC2 - 256*dist2(p,s) - j_s            (exact integers)
    via centered/scaled coords so every partial sum stays < 2^24 (bit-
    exact fp32; verified on HW). Larger nkey == nearer; ties broken by
    lower pixel index (j follows candidate order within the window).
  - 4 selection rounds on the Vector engine per block:
      m_r = reduce_max(nkey);  nkey = (nkey < m_r) * nkey
    (keys are distinct positive, so masking the extracted max to zero is
    safe; duplicated padding slots share one key and collapse together).
  - Output: the 4 winning keys per query, decoded on host into indices
    and offsets (pure elementwise output formatting).

Correctness for arbitrary inputs: after the run the host *proves* the
windowed result exact: the 4th-nearest squared distance returned for each
query must be strictly below the squared distance to the nearest grid row/
column not covered by the window rectangle. Any failing queries (none, for
typical 5%-density inputs) are recomputed exactly.
"""

import sys

sys.path.insert(0, "/opt/trn_rl_repo")

import numpy as np

B = 2
H = 64
W = 192
N = H * W            # 12288 pixels
NUM = 4
WIN = 48             # candidates per window (j fits in 8 bits)
GROUP_SIZES = (8, 8, 8)  # tiles per group (8*WIN == one PSUM bank)
KROWS = 14           # bf16 split-rows per tile (exact 8-bit-mantissa parts)
GK = KROWS * max(GROUP_SIZES)  # block-structured contraction dim per group
MM_SPLIT = 8           # tiles per matmul within a group (8*WIN <= PSUM bank)
KC2 = 10_356_000     # key bias: > 256*max_dist2 + 255, keeps partials < 2^24
N_CORES = 8
QPC = N // 4         # queries per core (4 cores per batch)
TILES = QPC // 128   # 24 blocks of 16x8 queries
BX, BY = 16, 8       # block extent in x / y
NBX = W // BX        # 12 blocks across
VALID_THRESH = 0.001

_PROGRAM = None


def _build_program():
    import concourse.bass as bass
    import concourse.bacc as bacc
    import concourse.mybir as mybir
    from concourse.tile import TileContext

    f32 = mybir.dt.float32
    ngroups = len(GROUP_SIZES)
    nc = bacc.Bacc(None)
    bf16 = mybir.dt.bfloat16
    lhsT_in = nc.declare_dram_parameter("lhsT_in", [GK, ngroups * 128], bf16, isOutput=False)
    rhs_in = nc.declare_dram_parameter("rhs_in", [GK, TILES * WIN], bf16, isOutput=False)
    m_out = nc.declare_dram_parameter("m_out", [128, TILES * NUM], f32, isOutput=True)

    with TileContext(nc) as tc:
        with (
            tc.tile_pool(name="const", bufs=1) as const,
            tc.tile_pool(name="work", bufs=2) as work,
            tc.tile_pool(name="psum", bufs=3, space="PSUM") as psum,
        ):
            lhsT_sb = const.tile([GK, ngroups * 128], bf16)
            rhs_sb = const.tile([GK, TILES * WIN], bf16)
            # mr layout: group-major, then round, then tile-in-group:
            # col = gbase*NUM + r*gsz + i
            mr = const.tile([128, TILES * NUM], f32)
            # group 0's inputs first (fast matmul start), then the rest
            # as single streaming transfers (fewer descriptor rounds)
            g0 = GROUP_SIZES[0]
            nc.sync.dma_start(out=rhs_sb[:, : g0 * WIN], in_=rhs_in[:, : g0 * WIN])
            nc.sync.dma_start(out=lhsT_sb[:, :128], in_=lhsT_in[:, :128])
            nc.sync.dma_start(out=rhs_sb[:, g0 * WIN :], in_=rhs_in[:, g0 * WIN :])
            nc.sync.dma_start(out=lhsT_sb[:, 128:], in_=lhsT_in[:, 128:])

            # shared contiguous chain tiles; rounds 0-1 run per group (they
            # overlap the DMA/matmul ramp), rounds 2-3 run once over all 24
            # tiles (amortized op overhead; by then the ramp is done).
            # mr layout: col = r*TILES + t
            AW = TILES * WIN
            nk1 = const.tile([128, AW], f32)
            nk2 = const.tile([128, AW], f32)
            nk3 = const.tile([128, AW], f32)
            msk = const.tile([128, AW], f32)

            def bcast(sl):
                return bass.AP(sl.tensor, sl.offset, list(sl.ap) + [[0, WIN]])

            def v3(tile_ap):
                return tile_ap.rearrange("p (t w) -> p t w", w=WIN)

            gb = 0
            for g, gsz in enumerate(GROUP_SIZES):
                assert gsz <= MM_SPLIT
                GW = gsz * WIN
                acc = psum.tile([128, GW], f32, tag="acc")
                nc.tensor.matmul(
                    acc[:, :],
                    lhsT_sb[:, g * 128 : (g + 1) * 128],
                    rhs_sb[:, gb * WIN : (gb + gsz) * WIN],
                    start=True,
                    stop=True,
                )
                w0, w1 = gb * WIN, (gb + gsz) * WIN
                # round 0 straight from PSUM (no copy)
                col = gb
                nc.vector.tensor_reduce(
                    out=mr[:, col : col + gsz],
                    in_=v3(acc[:, :]),
                    axis=mybir.AxisListType.X,
                    op=mybir.AluOpType.max,
                )
                nc.vector.tensor_tensor(
                    out=v3(msk[:, w0:w1]),
                    in0=v3(acc[:, :]),
                    in1=bcast(mr[:, col : col + gsz]),
                    op=mybir.AluOpType.is_lt,
                )
                nc.vector.tensor_tensor(
                    out=v3(nk1[:, w0:w1]),
                    in0=v3(msk[:, w0:w1]),
                    in1=v3(acc[:, :]),
                    op=mybir.AluOpType.mult,
                )
                gb += gsz

            # global round 1
            col = 1 * TILES
            nc.vector.tensor_reduce(
                out=mr[:, col : col + TILES], in_=v3(nk1[:, :]),
                axis=mybir.AxisListType.X, op=mybir.AluOpType.max,
            )
            nc.vector.tensor_tensor(
                out=v3(msk[:, :]), in0=v3(nk1[:, :]),
                in1=bcast(mr[:, col : col + TILES]), op=mybir.AluOpType.is_lt,
            )
            nc.vector.tensor_tensor(
                out=v3(nk2[:, :]), in0=v3(msk[:, :]), in1=v3(nk1[:, :]),
                op=mybir.AluOpType.mult,
            )

            # global rounds 2-3
            col = 2 * TILES
            nc.vector.tensor_reduce(
                out=mr[:, col : col + TILES], in_=v3(nk2[:, :]),
                axis=mybir.AxisListType.X, op=mybir.AluOpType.max,
            )
            nc.vector.tensor_tensor(
                out=v3(msk[:, :]), in0=v3(nk2[:, :]),
                in1=bcast(mr[:, col : col + TILES]), op=mybir.AluOpType.is_lt,
            )
            nc.vector.tensor_tensor(
                out=v3(nk3[:, :]), in0=v3(msk[:, :]), in1=v3(nk2[:, :]),
                op=mybir.AluOpType.mult,
            )
            col = 3 * TILES
            nc.vector.tensor_reduce(
                out=mr[:, col : col + TILES], in_=v3(nk3[:, :]),
                axis=mybir.AxisListType.X, op=mybir.AluOpType.max,
            )
            nc.sync.dma_start(out=m_out[:, :], in_=mr[:, :])

    nc.compile()
    return nc


def _split2(x, lo_bits):
    """x = hi + lo with both parts exact in bf16 (<=8-bit mantissa)."""
    x = np.asarray(x, dtype=np.int64)
    s = np.sign(x)
    hi = s * ((np.abs(x) >> lo_bits) << lo_bits)
    return hi.astype(np.float64), (x - hi).astype(np.float64)


def _split3(x, b1, b2):
    x = np.asarray(x, dtype=np.int64)
    s = np.sign(x)
    p1 = s * ((np.abs(x) >> b1) << b1)
    rem = x - p1
    s2 = np.sign(rem)
    p2 = s2 * ((np.abs(rem) >> b2) << b2)
    return p1.astype(np.float64), p2.astype(np.float64), (rem - p2).astype(np.float64)


def _block_origin(q, t):
    """Grid origin (x0, y0) of block t in row-quarter q."""
    return (t % NBX) * BX, 16 * q + (t // NBX) * BY


def _choose_rect(ps, x0, y0):
    """Largest centered rectangle [x0-R, x0+BX-1+R] x [y0-R, y0+BY-1+R]
    holding at most WIN candidates. ps = 2D prefix sum of the valid mask."""

    def count(R):
        xl, xh = max(x0 - R, 0), min(x0 + BX - 1 + R, W - 1)
        yl, yh = max(y0 - R, 0), min(y0 + BY - 1 + R, H - 1)
        return ps[yh + 1, xh + 1] - ps[yl, xh + 1] - ps[yh + 1, xl] + ps[yl, xl]

    for R in range(40, 3, -1):
        if count(R) <= WIN:
            return R
    return 4


def _exact_topk_host(qn, g, cx, cy):
    """Exact reference-equivalent top-NUM for query pixels qn (fallback for
    queries whose window guarantee failed; not used on typical inputs)."""
    qx = qn % W
    qy = qn // W
    d = (qx[:, None] - cx[None, :]) ** 2 + (qy[:, None] - cy[None, :]) ** 2
    order = np.argsort(d, axis=1, kind="stable")  # ties -> lower slot = lower g
    sel = order[:, :NUM]
    return g[sel]


def kernel(S, xx, yy, num):
    from concourse.bass_utils import run_bass_kernel_spmd

    global _PROGRAM
    S = np.asarray(S)
    num = int(num)
    assert num == NUM and S.shape == (B, 1, H, W)

    Sf = S.reshape(B, N)
    valid = Sf > VALID_THRESH

    g_all, cx_all, cy_all, ps_all = [], [], [], []
    for b in range(B):
        g = np.nonzero(valid[b])[0].astype(np.int64)
        if len(g) < NUM:
            raise NotImplementedError("fewer than 4 valid candidates")
        g_all.append(g)
        cx_all.append(g % W)
        cy_all.append(g // W)
        ps = np.zeros((H + 1, W + 1), dtype=np.int64)
        ps[1:, 1:] = valid[b].reshape(H, W).cumsum(0).cumsum(1)
        ps_all.append(ps)

    # block-local partition -> query offset maps
    px = np.arange(128) % BX
    py = np.arange(128) // BX

    # ---- host marshaling: per-core lhsT / windowed rhs tables ----
    in_maps = []
    winmap = np.zeros((N_CORES, TILES, WIN), dtype=np.int64)  # slot -> pixel
    rects = np.zeros((N_CORES, TILES, 4), dtype=np.int64)     # xl, xh, yl, yh
    # tile -> (group, index-in-group); tiles are consecutive within groups
    grp_of, idx_of = [], []
    for g, gsz in enumerate(GROUP_SIZES):
        grp_of += [g] * gsz
        idx_of += list(range(gsz))
    ngroups = len(GROUP_SIZES)

    for c in range(N_CORES):
        b, q = c // 4, c % 4
        g, cx, cy, ps = g_all[b], cx_all[b], cy_all[b], ps_all[b]
        lhsT = np.zeros((GK, ngroups * 128), dtype=np.float32)
        rhs = np.zeros((GK, TILES * WIN), dtype=np.float32)
        for t in range(TILES):
            x0, y0 = _block_origin(q, t)
            qx = x0 + px
            qy = y0 + py
            a = 16 * qx - 1528
            ay = 16 * qy - 504
            gi, ii = grp_of[t], idx_of[t]
            sl = slice(gi * 128, (gi + 1) * 128)
            v0a, v0b, v0c = _split3(-(a * a + ay * ay), 14, 6)
            ah, al = _split2(a, 3)
            ayh, ayl = _split2(ay, 2)
            lrows = [v0a, v0b, v0c, ah, ah, al, al, ayh, ayh, ayl, ayl,
                     np.ones(128), np.ones(128), np.ones(128)]
            for k, row in enumerate(lrows):
                lhsT[KROWS * ii + k, sl] = row

            R = _choose_rect(ps, x0, y0)
            xl, xh = max(x0 - R, 0), min(x0 + BX - 1 + R, W - 1)
            yl, yh = max(y0 - R, 0), min(y0 + BY - 1 + R, H - 1)
            rects[c, t] = (xl, xh, yl, yh)
            inside = (cx >= xl) & (cx <= xh) & (cy >= yl) & (cy <= yh)
            sel = np.nonzero(inside)[0]          # ascending == g order
            cnt = len(sel)
            src = sel[np.minimum(np.arange(WIN), max(cnt - 1, 0))] if cnt else None
            if cnt == 0:
                bx = np.zeros(WIN); by = np.zeros(WIN)
                jj = np.zeros(WIN); winmap[c, t] = 0
            else:
                bx = 16 * cx[src] - 1528
                by = 16 * cy[src] - 504
                jj = np.minimum(np.arange(WIN), cnt - 1)  # dups share j
                winmap[c, t] = g[src]
            ws = slice(t * WIN, (t + 1) * WIN)
            bh, bl = _split2(2 * bx, 4)
            byh, byl = _split2(2 * by, 2)
            v3a, v3b, v3c = _split3(KC2 - jj - bx * bx - by * by, 16, 8)
            on = np.ones(WIN)
            rrows = [on, on, on, bh, bl, bh, bl, byh, byl, byh, byl, v3a, v3b, v3c]
            for k, row in enumerate(rrows):
                rhs[KROWS * ii + k, ws] = row
        import ml_dtypes
        in_maps.append({"lhsT_in": lhsT.astype(ml_dtypes.bfloat16),
                        "rhs_in": rhs.astype(ml_dtypes.bfloat16)})

    if _PROGRAM is None:
        _PROGRAM = _build_program()
    globals()["_LAST_IN_MAPS"] = in_maps
    res = run_bass_kernel_spmd(_PROGRAM, in_maps, list(range(N_CORES)))

    # ---- decode on host (elementwise output formatting) ----
    args = np.zeros((B, NUM, N), dtype=np.int32)
    gsel = np.zeros((B, NUM, N), dtype=np.int64)
    dist4 = np.zeros((B, N), dtype=np.int64)
    for c in range(N_CORES):
        b, q = c // 4, c % 4
        m_raw = np.rint(res.results[c]["m_out"].astype(np.float64)).astype(np.int64)
        # device layout col = r*TILES + t  ->  [p, t, r]
        colmap = np.empty((TILES, NUM), dtype=np.int64)
        for t in range(TILES):
            for r in range(NUM):
                colmap[t, r] = r * TILES + t
        m = m_raw[:, colmap]
        key = KC2 - m                     # = 256*d + j
        d = key >> 8
        j = np.clip(key & 255, 0, WIN - 1)
        for t in range(TILES):
            x0, y0 = _block_origin(q, t)
            n_of = (y0 + py) * W + (x0 + px)      # [128] pixel index
            gg = winmap[c, t][j[:, t, :]]         # [128, NUM]
            for r in range(NUM):
                args[b, r, n_of] = gg[:, r].astype(np.int32)
                gsel[b, r, n_of] = gg[:, r]
            dist4[b, n_of] = d[:, t, NUM - 1]

    # ---- post-hoc exactness proof of the windowed result ----
    for c in range(N_CORES):
        b, q = c // 4, c % 4
        for t in range(TILES):
            xl, xh, yl, yh = rects[c, t]
            x0, y0 = _block_origin(q, t)
            qx = x0 + px
            qy = y0 + py
            n_of = qy * W + qx
            guard = np.full(128, 10**12, dtype=np.int64)
            if xl > 0:
                guard = np.minimum(guard, (qx - xl + 1) ** 2)
            if xh < W - 1:
                guard = np.minimum(guard, (xh + 1 - qx) ** 2)
            if yl > 0:
                guard = np.minimum(guard, (qy - yl + 1) ** 2)
            if yh < H - 1:
                guard = np.minimum(guard, (yh + 1 - qy) ** 2)
            bad = dist4[b, n_of] >= guard
            if bad.any():
                qbad = n_of[bad]
                gsel_f = _exact_topk_host(qbad, g_all[b], cx_all[b], cy_all[b])
                args[b, :, qbad] = gsel_f.astype(np.int32)
                gsel[b, :, qbad] = gsel_f

    # IPC offsets from the input coordinate grids (matches reference's
    # xy[:, args] - xy[:, query])
    xf = np.asarray(xx, dtype=np.float32).reshape(N)
    yf = np.asarray(yy, dtype=np.float32).reshape(N)
    IPCnum = np.empty((B, 2, NUM, N), dtype=np.float32)
    for b in range(B):
        IPCnum[b, 0] = xf[gsel[b]] - xf[None, :]
        IPCnum[b, 1] = yf[gsel[b]] - yf[None, :]

    return IPCnum, args
